# revision 14
# baseline (speedup 1.0000x reference)
"""AFM layer kernel for 8 Trainium2 NeuronCores.

Math (per batch b, F=50 fields, E=64, A=10):
  pairs p=(i<j), inter_p = x_i * x_j
  q_c[p]  = inter_p . W[:,c],  logit l_p = sum_c h_c relu(q_c[p] + b_c)
  score = softmax_p(l),  out[b] = sum_p score_p * (inter_p . proj_p)

Reformulation: out[b] = (sum_p e_p r_p) / (sum_p e_p) with e_p = exp(l_p),
r_p = inter_p . proj_p.  Both q_c and r are bilinear forms
x_i^T diag(c) x_j, so inter [B,1225,64] is never materialized.

Per 2-batch iteration (bf16 matmul path, f32 accumulation):
  Q[(half,i), (c,j)] = st^T @ u  (+ bias via K=1 accumulating matmul)
    st = block-diag(x_b1^T, x_b2^T) [128,100]   (prepacked on host)
    u[(half,e),(c,j)] = xt[(half,e),j] * Cmat[e,c]  [128,550]
      (xt = stacked batch transposes [128,50], prepacked on host;
       Cmat = [|h|-scaled sign-ordered W columns | projection_p])
  relu -> signed c-sum (strided reduces) -> +mask (kill i>=j pairs via
  -30 logit offset) -> exp -> num/den per (half,i) accumulated into acc;
  final K=100 matmul sums over i, reciprocal+mul -> out [256,1].

|h| and sign(h) are folded into W/bias on the host; columns reordered so
positive-sign c's are contiguous (two strided reduces + subtract).
"""

import os
import sys
import numpy as np

for _p in ("/opt/trn_rl_repo",):
    if _p not in sys.path:
        sys.path.insert(0, _p)

B = 2048
NCORES = 8
B_LOC = B // NCORES  # 256
NF = 50
E = 64
NA = 10
NCOL = NA + 1  # 10 W-columns + projection_p
NIT = B_LOC // 2  # 128 two-batch iterations
MASK_NEG = -30.0
W5 = NCOL * NF  # 550

LAST_RESULTS = None  # stash for test.py (exec_time_ns etc.)


def _build(npos, nneg):
    from contextlib import ExitStack
    import concourse.bass as bass
    import concourse.tile as tile
    from concourse import bacc, mybir

    f32 = mybir.dt.float32
    bf16 = mybir.dt.bfloat16
    AF = mybir.ActivationFunctionType
    OP = mybir.AluOpType

    nc = bacc.Bacc()
    xt_ext = nc.declare_dram_parameter("xt", [128, NIT * NF], bf16, isOutput=False)
    std_ext = nc.declare_dram_parameter("std", [128, NIT * 100], bf16, isOutput=False)
    cexp_ext = nc.declare_dram_parameter("cexp", [128, W5], bf16, isOutput=False)
    cbias_ext = nc.declare_dram_parameter("cbias", [1, W5], bf16, isOutput=False)
    maskt_ext = nc.declare_dram_parameter("maskt", [100, NF], bf16, isOutput=False)
    onesfin_ext = nc.declare_dram_parameter("onesfin", [100, 2], f32, isOutput=False)
    out_ext = nc.declare_dram_parameter("out", [B_LOC, 1], f32, isOutput=True)

    with tile.TileContext(nc) as tc, ExitStack() as ctx:
        cpool = ctx.enter_context(tc.tile_pool(name="const", bufs=1))
        xpool = ctx.enter_context(tc.tile_pool(name="xin", bufs=4))
        upool = ctx.enter_context(tc.tile_pool(name="u", bufs=3))
        relp = ctx.enter_context(tc.tile_pool(name="rel", bufs=3))
        smp = ctx.enter_context(tc.tile_pool(name="small", bufs=6))
        accp = ctx.enter_context(tc.tile_pool(name="acc", bufs=1))
        psq = ctx.enter_context(tc.tile_pool(name="psq", bufs=3, space="PSUM"))
        psf = ctx.enter_context(tc.tile_pool(name="psf", bufs=1, space="PSUM"))

        # ---- constants ----
        cexp = cpool.tile([128, W5], bf16)
        nc.sync.dma_start(cexp[:], cexp_ext[:])
        cbias = cpool.tile([1, W5], bf16)
        nc.sync.dma_start(cbias[:], cbias_ext[:])
        maskt = cpool.tile([100, NF], bf16)
        nc.sync.dma_start(maskt[:], maskt_ext[:])
        onesfin = cpool.tile([100, 2], f32)
        nc.sync.dma_start(onesfin[:], onesfin_ext[:])
        ones_st = cpool.tile([1, 100], bf16)
        nc.vector.memset(ones_st[:], 1.0)

        acc = accp.tile([100, 2 * NIT], f32)

        for it in range(NIT):
            xt_t = xpool.tile([128, NF], bf16, tag="xt_t")
            nc.sync.dma_start(xt_t[:], xt_ext[:, it * NF : (it + 1) * NF])
            st_t = xpool.tile([128, 100], bf16, tag="st_t")
            nc.sync.dma_start(st_t[:], std_ext[:, it * 100 : (it + 1) * 100])

            # u[(half,e), (c,j)] = xt[(half,e), j] * cexp[(half,e), (c,j)]
            u = upool.tile([128, W5], bf16)
            xt_b = xt_t[:].unsqueeze(1).broadcast_to([128, NCOL, NF])
            nc.vector.tensor_tensor(
                u[:].rearrange("p (c j) -> p c j", j=NF),
                xt_b,
                cexp[:].rearrange("p (c j) -> p c j", j=NF),
                op=OP.mult,
            )

            qA = psq.tile([100, 300], f32, tag="qA")
            qB = psq.tile([100, 250], f32, tag="qB")
            nc.tensor.matmul(qA[:], ones_st[:], cbias[:, 0:300], start=True, stop=False)
            nc.tensor.matmul(qA[:], st_t[:], u[:, 0:300], start=False, stop=True)
            nc.tensor.matmul(qB[:], ones_st[:], cbias[:, 300:W5], start=True, stop=False)
            nc.tensor.matmul(qB[:], st_t[:], u[:, 300:W5], start=False, stop=True)

            # relu over the 10 W-columns (cols 0:500); col-block 10 (500:550) is r
            rel = relp.tile([100, NA * NF], bf16, tag="rel")
            nc.scalar.activation(rel[:, 0:300], qA[:], AF.Relu)
            nc.scalar.activation(rel[:, 300:500], qB[:, 0:200], AF.Relu)
            rr = relp.tile([100, NF], bf16, tag="rr")
            nc.scalar.copy(rr[:], qB[:, 200:250])

            # signed c-sum via pairwise TT-add trees (contiguous APs only --
            # strided multi-dim reduce and accum_out fault TRN2 hardware)
            def tree_sum(nblk, base, tag):
                if nblk == 0:
                    z = smp.tile([100, NF], bf16, tag=f"{tag}z", name=f"{tag}z")
                    nc.vector.memset(z[:], 0.0)
                    return z[:]
                cur = [
                    rel[:, (base + i) * NF : (base + i + 1) * NF]
                    for i in range(nblk)
                ]
                k = 0
                while len(cur) > 1:
                    nxt = []
                    for i in range(0, len(cur) - 1, 2):
                        t = smp.tile(
                            [100, NF], bf16, tag=f"{tag}{k}", name=f"{tag}{k}"
                        )
                        k += 1
                        nc.vector.tensor_tensor(t[:], cur[i], cur[i + 1], op=OP.add)
                        nxt.append(t[:])
                    if len(cur) % 2:
                        nxt.append(cur[-1])
                    cur = nxt
                return cur[0]

            lp = tree_sum(npos, 0, "lp")
            ln = tree_sum(nneg, npos, "ln")
            ell = smp.tile([100, NF], bf16, tag="ell")
            nc.vector.tensor_tensor(ell[:], lp, ln, op=OP.subtract)
            ellm = smp.tile([100, NF], bf16, tag="ellm")
            nc.vector.tensor_tensor(ellm[:], ell[:], maskt[:], op=OP.add)

            e_t = smp.tile([100, NF], bf16, tag="et")
            nc.scalar.activation(e_t[:], ellm[:], AF.Exp)
            nc.vector.reduce_sum(
                acc[:, 2 * it + 1 : 2 * it + 2], e_t[:], axis=mybir.AxisListType.X
            )
            ers = smp.tile([100, NF], bf16, tag="ers")
            nc.vector.tensor_tensor(ers[:], e_t[:], rr[:], op=OP.mult)
            nc.vector.reduce_sum(
                acc[:, 2 * it : 2 * it + 1], ers[:], axis=mybir.AxisListType.X
            )

        # final: per-batch partition sums (num, den), divide, store
        pf = psf.tile([2, 2 * NIT], f32)
        nc.tensor.matmul(pf[:], onesfin[:], acc[:], start=True, stop=True)
        accv = pf[:].rearrange("p (i t) -> p i t", t=2)
        rcp = smp.tile([2, NIT], f32, tag="rcp")
        nc.vector.reciprocal(rcp[:], accv[:, :, 1])
        res = smp.tile([2, NIT], f32, tag="res")
        nc.vector.tensor_tensor(res[:], accv[:, :, 0], rcp[:], op=OP.mult)
        out_v = out_ext[:].rearrange("(i m) o -> m (i o)", m=2)
        nc.sync.dma_start(out_v, res[:])

    nc.compile()
    return nc


def _host_prep(x, attention_W, attention_b, projection_h, projection_p):
    import ml_dtypes

    bf = ml_dtypes.bfloat16
    x = np.ascontiguousarray(np.asarray(x, dtype=np.float32))
    W = np.asarray(attention_W, dtype=np.float32)
    bv = np.asarray(attention_b, dtype=np.float32)
    h = np.asarray(projection_h, dtype=np.float32).reshape(-1)
    p = np.asarray(projection_p, dtype=np.float32).reshape(-1)

    habs = np.abs(h)
    sgn = np.sign(h)
    order = np.argsort(-sgn, kind="stable")
    Wp = (W * habs[None, :])[:, order]
    bp = (bv * habs)[order]
    npos = int((sgn[order] > 0).sum())

    Cmat = np.concatenate([Wp, p[:, None]], axis=1)  # [64, 11]
    cexp64 = np.broadcast_to(Cmat.T[:, None, :], (NCOL, NF, E))  # [11,50,64]
    cexp64 = np.transpose(cexp64, (2, 0, 1)).reshape(E, W5)  # [64,550]
    cexp = np.concatenate([cexp64, cexp64], axis=0).astype(bf)  # [128,550]
    cbias = (
        np.concatenate([np.repeat(bp, NF), np.zeros(NF)]).reshape(1, -1).astype(bf)
    )
    im = np.arange(NF)
    m50 = np.where(im[:, None] < im[None, :], 0.0, MASK_NEG).astype(bf)
    maskt = np.ascontiguousarray(np.concatenate([m50, m50], axis=0))  # [100, 50]
    onesfin = np.zeros((100, 2), dtype=np.float32)
    onesfin[0:50, 0] = 1.0
    onesfin[50:100, 1] = 1.0

    # per-core packed layouts
    xcores = x.reshape(NCORES, B_LOC, NF, E)
    xt_bf = np.transpose(xcores, (0, 1, 3, 2)).astype(bf)  # [C, 256, 64, 50]
    xt_it = xt_bf.reshape(NCORES, NIT, 2, E, NF)  # [C, it, half, e, j]
    # xt [128, NIT*50]: rows (half, e), cols (it, j)
    xt_all = np.ascontiguousarray(
        xt_it.transpose(0, 2, 3, 1, 4).reshape(NCORES, 128, NIT * NF)
    )
    # std [128, NIT*100]: block-diag per iteration
    std_all = np.zeros((NCORES, 2, E, NIT, 2, NF), dtype=bf)  # [C,rh,e,it,ch,j]
    std_all[:, 0, :, :, 0, :] = xt_it[:, :, 0].transpose(0, 2, 1, 3)
    std_all[:, 1, :, :, 1, :] = xt_it[:, :, 1].transpose(0, 2, 1, 3)
    std_all = np.ascontiguousarray(std_all.reshape(NCORES, 128, NIT * 100))

    return npos, xt_all, std_all, cexp, cbias, maskt, onesfin


def kernel(x, attention_W, attention_b, projection_h, projection_p):
    global LAST_RESULTS
    from concourse.bass_utils import run_bass_kernel_spmd

    npos, xt_all, std_all, cexp, cbias, maskt, onesfin = _host_prep(
        x, attention_W, attention_b, projection_h, projection_p
    )
    nc = _build(npos, NA - npos)

    in_maps = []
    for c in range(NCORES):
        in_maps.append(
            {
                "xt": xt_all[c],
                "std": std_all[c],
                "cexp": cexp,
                "cbias": cbias,
                "maskt": maskt,
                "onesfin": onesfin,
            }
        )
    trace = os.environ.get("BASS_KERNEL_TRACE", "0") == "1"
    res = run_bass_kernel_spmd(nc, in_maps, core_ids=list(range(NCORES)), trace=trace)
    LAST_RESULTS = res
    outs = [np.asarray(r["out"]).reshape(B_LOC, 1) for r in res.results]
    return np.concatenate(outs, axis=0).astype(np.float32)


# revision 16
# speedup vs baseline: 1.6356x; 1.6356x over previous
"""AFM layer kernel for 8 Trainium2 NeuronCores.

Math (per batch b, F=50 fields, E=64, A=10):
  pairs p=(i<j), inter_p = x_i * x_j
  q_c[p]  = inter_p . W[:,c],  logit l_p = sum_c h_c relu(q_c[p] + b_c)
  score = softmax_p(l),  out[b] = sum_p score_p * (inter_p . proj_p)

Reformulation: out[b] = (sum_p e_p r_p) / (sum_p e_p) with e_p = exp(l_p),
r_p = inter_p . proj_p.  Both q_c and r are bilinear forms
x_i^T diag(c) x_j, so inter [B,1225,64] is never materialized.

Per 2-batch iteration (bf16 matmul path, f32 accumulation):
  Q[(half,i), (c,j)] = st^T @ u  (+ bias via K=1 accumulating matmul)
    st = block-diag(x_b1^T, x_b2^T) [128,100]   (prepacked on host)
    u[(half,e),(c,j)] = xt[(half,e),j] * Cmat[e,c]  [128,550]
      (xt = stacked batch transposes [128,50], prepacked on host;
       Cmat = [|h|-scaled sign-ordered W columns | projection_p])
  relu -> signed c-sum (strided reduces) -> +mask (kill i>=j pairs via
  -30 logit offset) -> exp -> num/den per (half,i) accumulated into acc;
  final K=100 matmul sums over i, reciprocal+mul -> out [256,1].

|h| and sign(h) are folded into W/bias on the host; columns reordered so
positive-sign c's are contiguous (two strided reduces + subtract).
"""

import os
import sys
import numpy as np

for _p in ("/opt/trn_rl_repo",):
    if _p not in sys.path:
        sys.path.insert(0, _p)

B = 2048
NCORES = 8
B_LOC = B // NCORES  # 256
NF = 50
E = 64
NA = 10
NCOL = NA + 1  # 10 W-columns + projection_p
NIT = B_LOC // 2  # 128 two-batch iterations
MASK_NEG = -30.0
W5 = NCOL * NF  # 550

LAST_RESULTS = None  # stash for test.py (exec_time_ns etc.)


def _build(npos, nneg, rep=1):
    from contextlib import ExitStack
    import concourse.bass as bass
    import concourse.tile as tile
    from concourse import bacc, mybir

    f32 = mybir.dt.float32
    bf16 = mybir.dt.bfloat16
    AF = mybir.ActivationFunctionType
    OP = mybir.AluOpType

    nc = bacc.Bacc()
    xt_ext = nc.declare_dram_parameter("xt", [128, NIT * NF], bf16, isOutput=False)
    std_ext = nc.declare_dram_parameter("std", [128, NIT * 100], bf16, isOutput=False)
    cexp_ext = nc.declare_dram_parameter("cexp", [128, W5], bf16, isOutput=False)
    cbias_ext = nc.declare_dram_parameter("cbias", [1, W5], bf16, isOutput=False)
    maskt_ext = nc.declare_dram_parameter("maskt", [100, NF], bf16, isOutput=False)
    onesfin_ext = nc.declare_dram_parameter("onesfin", [100, 2], f32, isOutput=False)
    out_ext = nc.declare_dram_parameter("out", [B_LOC, 1], f32, isOutput=True)

    with tile.TileContext(nc) as tc, ExitStack() as ctx:
        cpool = ctx.enter_context(tc.tile_pool(name="const", bufs=1))
        xpool = ctx.enter_context(tc.tile_pool(name="xin", bufs=4))
        upool = ctx.enter_context(tc.tile_pool(name="u", bufs=3))
        relp = ctx.enter_context(tc.tile_pool(name="rel", bufs=3))
        smp = ctx.enter_context(tc.tile_pool(name="small", bufs=6))
        accp = ctx.enter_context(tc.tile_pool(name="acc", bufs=1))
        psq = ctx.enter_context(tc.tile_pool(name="psq", bufs=3, space="PSUM"))
        psf = ctx.enter_context(tc.tile_pool(name="psf", bufs=1, space="PSUM"))

        # ---- constants ----
        cexp = cpool.tile([128, W5], bf16)
        nc.sync.dma_start(cexp[:], cexp_ext[:])
        cbias = cpool.tile([1, W5], bf16)
        nc.sync.dma_start(cbias[:], cbias_ext[:])
        maskt = cpool.tile([100, NF], bf16)
        nc.sync.dma_start(maskt[:], maskt_ext[:])
        onesfin = cpool.tile([100, 2], f32)
        nc.sync.dma_start(onesfin[:], onesfin_ext[:])
        ones_st = cpool.tile([1, 100], bf16)
        nc.vector.memset(ones_st[:], 1.0)

        acc = accp.tile([100, 2 * NIT], f32)

        for rit in range(rep * NIT):
            it = rit % NIT
            k4 = rit % 4  # slab slot
            xt_t = xpool.tile([128, NF], bf16, tag="xt_t")
            nc.sync.dma_start(xt_t[:], xt_ext[:, it * NF : (it + 1) * NF])
            st_t = xpool.tile([128, 100], bf16, tag="st_t")
            nc.sync.dma_start(st_t[:], std_ext[:, it * 100 : (it + 1) * 100])

            # u[(half,e), (c,j)] = xt[(half,e), j] * cexp[(half,e), (c,j)]
            u = upool.tile([128, W5], bf16)
            xt_b = xt_t[:].unsqueeze(1).broadcast_to([128, NCOL, NF])
            nc.vector.tensor_tensor(
                u[:].rearrange("p (c j) -> p c j", j=NF),
                xt_b,
                cexp[:].rearrange("p (c j) -> p c j", j=NF),
                op=OP.mult,
            )

            qA = psq.tile([100, 300], f32, tag="qA")
            qB = psq.tile([100, 250], f32, tag="qB")
            nc.tensor.matmul(qA[:], ones_st[:], cbias[:, 0:300], start=True, stop=False)
            nc.tensor.matmul(qA[:], st_t[:], u[:, 0:300], start=False, stop=True)
            nc.tensor.matmul(qB[:], ones_st[:], cbias[:, 300:W5], start=True, stop=False)
            nc.tensor.matmul(qB[:], st_t[:], u[:, 300:W5], start=False, stop=True)

            # relu into the current 4-slab slot; col-block 10 is r
            if k4 == 0:
                rel4 = relp.tile([100, 4 * NA * NF], bf16, tag="rel4")
                rr4 = relp.tile([100, 4 * NF], bf16, tag="rr4")
            rb = k4 * NA * NF
            nc.scalar.activation(rel4[:, rb : rb + 300], qA[:], AF.Relu)
            nc.scalar.activation(rel4[:, rb + 300 : rb + 500], qB[:, 0:200], AF.Relu)
            nc.scalar.copy(rr4[:, k4 * NF : (k4 + 1) * NF], qB[:, 200:250])

            if k4 != 3:
                continue

            # ---- slab-batched downstream over 4 iterations ----
            base_it = it - 3
            r4v = rel4[:].rearrange("p (k cj) -> p k cj", k=4)

            def blk(i):
                return r4v[:, :, i * NF : (i + 1) * NF]  # [100, 4, 50]

            def tree_sum(nblk, base, tag):
                if nblk == 0:
                    z = smp.tile([100, 4 * NF], bf16, tag=f"{tag}z", name=f"{tag}z")
                    nc.vector.memset(z[:], 0.0)
                    return z[:].rearrange("p (k j) -> p k j", k=4)
                cur = [blk(base + i) for i in range(nblk)]
                k = 0
                while len(cur) > 1:
                    nxt = []
                    for i in range(0, len(cur) - 1, 2):
                        t = smp.tile(
                            [100, 4 * NF], bf16, tag=f"{tag}{k}", name=f"{tag}{k}"
                        )
                        k += 1
                        tv = t[:].rearrange("p (k j) -> p k j", k=4)
                        nc.vector.tensor_tensor(tv, cur[i], cur[i + 1], op=OP.add)
                        nxt.append(tv)
                    if len(cur) % 2:
                        nxt.append(cur[-1])
                    cur = nxt
                return cur[0]

            lp = tree_sum(npos, 0, "lp")
            ln = tree_sum(nneg, npos, "ln")
            ell = smp.tile([100, 4 * NF], bf16, tag="ell")
            ellv = ell[:].rearrange("p (k j) -> p k j", k=4)
            nc.vector.tensor_tensor(ellv, lp, ln, op=OP.subtract)
            ellm = smp.tile([100, 4 * NF], bf16, tag="ellm")
            ellmv = ellm[:].rearrange("p (k j) -> p k j", k=4)
            mb = maskt[:].unsqueeze(1).broadcast_to([100, 4, NF])
            nc.vector.tensor_tensor(ellmv, ellv, mb, op=OP.add)

            e_t = smp.tile([100, 4 * NF], bf16, tag="et")
            nc.scalar.activation(e_t[:], ellm[:], AF.Exp)
            ers = smp.tile([100, 4 * NF], bf16, tag="ers")
            nc.vector.tensor_tensor(ers[:], e_t[:], rr4[:], op=OP.mult)
            # per-k contiguous reduces (strided/multi-dim reduce faults HW)
            for kk in range(4):
                nc.vector.reduce_sum(
                    acc[:, NIT + base_it + kk : NIT + base_it + kk + 1],
                    e_t[:, kk * NF : (kk + 1) * NF],
                    axis=mybir.AxisListType.X,
                )
                nc.vector.reduce_sum(
                    acc[:, base_it + kk : base_it + kk + 1],
                    ers[:, kk * NF : (kk + 1) * NF],
                    axis=mybir.AxisListType.X,
                )

        # final: per-batch partition sums (num, den), divide, store
        pf = psf.tile([2, 2 * NIT], f32)
        nc.tensor.matmul(pf[:], onesfin[:], acc[:], start=True, stop=True)
        rcp = smp.tile([2, NIT], f32, tag="rcp")
        nc.vector.reciprocal(rcp[:], pf[:, NIT : 2 * NIT])
        res = smp.tile([2, NIT], f32, tag="res")
        nc.vector.tensor_tensor(res[:], pf[:, 0:NIT], rcp[:], op=OP.mult)
        out_v = out_ext[:].rearrange("(i m) o -> m (i o)", m=2)
        nc.sync.dma_start(out_v, res[:])

    nc.compile()
    return nc


def _host_prep(x, attention_W, attention_b, projection_h, projection_p):
    import ml_dtypes

    bf = ml_dtypes.bfloat16
    x = np.ascontiguousarray(np.asarray(x, dtype=np.float32))
    W = np.asarray(attention_W, dtype=np.float32)
    bv = np.asarray(attention_b, dtype=np.float32)
    h = np.asarray(projection_h, dtype=np.float32).reshape(-1)
    p = np.asarray(projection_p, dtype=np.float32).reshape(-1)

    habs = np.abs(h)
    sgn = np.sign(h)
    order = np.argsort(-sgn, kind="stable")
    Wp = (W * habs[None, :])[:, order]
    bp = (bv * habs)[order]
    npos = int((sgn[order] > 0).sum())

    Cmat = np.concatenate([Wp, p[:, None]], axis=1)  # [64, 11]
    cexp64 = np.broadcast_to(Cmat.T[:, None, :], (NCOL, NF, E))  # [11,50,64]
    cexp64 = np.transpose(cexp64, (2, 0, 1)).reshape(E, W5)  # [64,550]
    cexp = np.concatenate([cexp64, cexp64], axis=0).astype(bf)  # [128,550]
    cbias = (
        np.concatenate([np.repeat(bp, NF), np.zeros(NF)]).reshape(1, -1).astype(bf)
    )
    im = np.arange(NF)
    m50 = np.where(im[:, None] < im[None, :], 0.0, MASK_NEG).astype(bf)
    maskt = np.ascontiguousarray(np.concatenate([m50, m50], axis=0))  # [100, 50]
    onesfin = np.zeros((100, 2), dtype=np.float32)
    onesfin[0:50, 0] = 1.0
    onesfin[50:100, 1] = 1.0

    # per-core packed layouts
    xcores = x.reshape(NCORES, B_LOC, NF, E)
    xt_bf = np.transpose(xcores, (0, 1, 3, 2)).astype(bf)  # [C, 256, 64, 50]
    xt_it = xt_bf.reshape(NCORES, NIT, 2, E, NF)  # [C, it, half, e, j]
    # xt [128, NIT*50]: rows (half, e), cols (it, j)
    xt_all = np.ascontiguousarray(
        xt_it.transpose(0, 2, 3, 1, 4).reshape(NCORES, 128, NIT * NF)
    )
    # std [128, NIT*100]: block-diag per iteration
    std_all = np.zeros((NCORES, 2, E, NIT, 2, NF), dtype=bf)  # [C,rh,e,it,ch,j]
    std_all[:, 0, :, :, 0, :] = xt_it[:, :, 0].transpose(0, 2, 1, 3)
    std_all[:, 1, :, :, 1, :] = xt_it[:, :, 1].transpose(0, 2, 1, 3)
    std_all = np.ascontiguousarray(std_all.reshape(NCORES, 128, NIT * 100))

    return npos, xt_all, std_all, cexp, cbias, maskt, onesfin


def kernel(x, attention_W, attention_b, projection_h, projection_p):
    global LAST_RESULTS
    from concourse.bass_utils import run_bass_kernel_spmd

    npos, xt_all, std_all, cexp, cbias, maskt, onesfin = _host_prep(
        x, attention_W, attention_b, projection_h, projection_p
    )
    nc = _build(npos, NA - npos)

    in_maps = []
    for c in range(NCORES):
        in_maps.append(
            {
                "xt": xt_all[c],
                "std": std_all[c],
                "cexp": cexp,
                "cbias": cbias,
                "maskt": maskt,
                "onesfin": onesfin,
            }
        )
    trace = os.environ.get("BASS_KERNEL_TRACE", "0") == "1"
    res = run_bass_kernel_spmd(nc, in_maps, core_ids=list(range(NCORES)), trace=trace)
    LAST_RESULTS = res
    outs = [np.asarray(r["out"]).reshape(B_LOC, 1) for r in res.results]
    return np.concatenate(outs, axis=0).astype(np.float32)


# revision 17
# speedup vs baseline: 1.6752x; 1.0242x over previous
"""AFM layer kernel for 8 Trainium2 NeuronCores.

Math (per batch b, F=50 fields, E=64, A=10):
  pairs p=(i<j), inter_p = x_i * x_j
  q_c[p]  = inter_p . W[:,c],  logit l_p = sum_c h_c relu(q_c[p] + b_c)
  score = softmax_p(l),  out[b] = sum_p score_p * (inter_p . proj_p)

Reformulation: out[b] = (sum_p e_p r_p) / (sum_p e_p) with e_p = exp(l_p),
r_p = inter_p . proj_p.  Both q_c and r are bilinear forms
x_i^T diag(c) x_j, so inter [B,1225,64] is never materialized.

Per 2-batch iteration (bf16 matmul path, f32 accumulation):
  Q[(half,i), (c,j)] = st^T @ u  (+ bias via K=1 accumulating matmul)
    st = block-diag(x_b1^T, x_b2^T) [128,100]   (prepacked on host)
    u[(half,e),(c,j)] = xt[(half,e),j] * Cmat[e,c]  [128,550]
      (xt = stacked batch transposes [128,50], prepacked on host;
       Cmat = [|h|-scaled sign-ordered W columns | projection_p])
  relu -> signed c-sum (strided reduces) -> +mask (kill i>=j pairs via
  -30 logit offset) -> exp -> num/den per (half,i) accumulated into acc;
  final K=100 matmul sums over i, reciprocal+mul -> out [256,1].

|h| and sign(h) are folded into W/bias on the host; columns reordered so
positive-sign c's are contiguous (two strided reduces + subtract).
"""

import os
import sys
import numpy as np

for _p in ("/opt/trn_rl_repo",):
    if _p not in sys.path:
        sys.path.insert(0, _p)

B = 2048
NCORES = 8
B_LOC = B // NCORES  # 256
NF = 50
E = 64
NA = 10
NCOL = NA + 1  # 10 W-columns + projection_p
NIT = B_LOC // 2  # 128 two-batch iterations
MASK_NEG = -30.0
W5 = NCOL * NF  # 550

LAST_RESULTS = None  # stash for test.py (exec_time_ns etc.)


def _build(npos, nneg, rep=1):
    from contextlib import ExitStack
    import concourse.bass as bass
    import concourse.tile as tile
    from concourse import bacc, mybir

    f32 = mybir.dt.float32
    bf16 = mybir.dt.bfloat16
    AF = mybir.ActivationFunctionType
    OP = mybir.AluOpType

    nc = bacc.Bacc()
    xt_ext = nc.declare_dram_parameter("xt", [128, NIT * NF], bf16, isOutput=False)
    std_ext = nc.declare_dram_parameter("std", [128, NIT * 100], bf16, isOutput=False)
    cexp_ext = nc.declare_dram_parameter("cexp", [128, W5], bf16, isOutput=False)
    cbias_ext = nc.declare_dram_parameter("cbias", [1, W5], bf16, isOutput=False)
    maskt_ext = nc.declare_dram_parameter("maskt", [100, NF], bf16, isOutput=False)
    onesfin_ext = nc.declare_dram_parameter("onesfin", [100, 2], f32, isOutput=False)
    out_ext = nc.declare_dram_parameter("out", [B_LOC, 1], f32, isOutput=True)

    with tile.TileContext(nc) as tc, ExitStack() as ctx:
        cpool = ctx.enter_context(tc.tile_pool(name="const", bufs=1))
        xpool = ctx.enter_context(tc.tile_pool(name="xin", bufs=4))
        upool = ctx.enter_context(tc.tile_pool(name="u", bufs=3))
        relp = ctx.enter_context(tc.tile_pool(name="rel", bufs=3))
        smp = ctx.enter_context(tc.tile_pool(name="small", bufs=6))
        accp = ctx.enter_context(tc.tile_pool(name="acc", bufs=1))
        psq = ctx.enter_context(tc.tile_pool(name="psq", bufs=3, space="PSUM"))
        psf = ctx.enter_context(tc.tile_pool(name="psf", bufs=1, space="PSUM"))

        # ---- constants ----
        cexp = cpool.tile([128, W5], bf16)
        nc.sync.dma_start(cexp[:], cexp_ext[:])
        cbias = cpool.tile([1, W5], bf16)
        nc.sync.dma_start(cbias[:], cbias_ext[:])
        maskt = cpool.tile([100, NF], bf16)
        nc.sync.dma_start(maskt[:], maskt_ext[:])
        onesfin = cpool.tile([100, 2], f32)
        nc.sync.dma_start(onesfin[:], onesfin_ext[:])
        ones_st = cpool.tile([1, 100], bf16)
        nc.vector.memset(ones_st[:], 1.0)

        acc = accp.tile([100, 2 * NIT], f32)

        for rit in range(rep * NIT):
            it = rit % NIT
            k4 = rit % 4  # slab slot
            xt_t = xpool.tile([128, NF], bf16, tag="xt_t")
            nc.sync.dma_start(xt_t[:], xt_ext[:, it * NF : (it + 1) * NF])
            st_t = xpool.tile([128, 100], bf16, tag="st_t")
            nc.sync.dma_start(st_t[:], std_ext[:, it * 100 : (it + 1) * 100])

            # u[(half,e), (c,j)] = xt[(half,e), j] * cexp[(half,e), (c,j)]
            u = upool.tile([128, W5], bf16)
            xt_b = xt_t[:].unsqueeze(1).broadcast_to([128, NCOL, NF])
            nc.vector.tensor_tensor(
                u[:].rearrange("p (c j) -> p c j", j=NF),
                xt_b,
                cexp[:].rearrange("p (c j) -> p c j", j=NF),
                op=OP.mult,
            )

            qA = psq.tile([100, 300], f32, tag="qA")
            qB = psq.tile([100, 250], f32, tag="qB")
            nc.tensor.matmul(qA[:], ones_st[:], cbias[:, 0:300], start=True, stop=False)
            nc.tensor.matmul(qA[:], st_t[:], u[:, 0:300], start=False, stop=True)
            nc.tensor.matmul(qB[:], ones_st[:], cbias[:, 300:W5], start=True, stop=False)
            nc.tensor.matmul(qB[:], st_t[:], u[:, 300:W5], start=False, stop=True)

            # relu into the current 4-slab slot; col-block 10 is r
            if k4 == 0:
                rel4 = relp.tile([100, 4 * NA * NF], bf16, tag="rel4")
                rr4 = relp.tile([100, 4 * NF], bf16, tag="rr4")
            rb = k4 * NA * NF
            nc.scalar.activation(rel4[:, rb : rb + 300], qA[:], AF.Relu)
            nc.scalar.activation(rel4[:, rb + 300 : rb + 500], qB[:, 0:200], AF.Relu)
            nc.vector.tensor_copy(rr4[:, k4 * NF : (k4 + 1) * NF], qB[:, 200:250])

            if k4 != 3:
                continue

            # ---- slab-batched downstream over 4 iterations ----
            base_it = it - 3
            r4v = rel4[:].rearrange("p (k cj) -> p k cj", k=4)

            def blk(i):
                return r4v[:, :, i * NF : (i + 1) * NF]  # [100, 4, 50]

            def tree_sum(nblk, base, tag):
                if nblk == 0:
                    z = smp.tile([100, 4 * NF], bf16, tag=f"{tag}z", name=f"{tag}z")
                    nc.vector.memset(z[:], 0.0)
                    return z[:].rearrange("p (k j) -> p k j", k=4)
                cur = [blk(base + i) for i in range(nblk)]
                k = 0
                while len(cur) > 1:
                    nxt = []
                    for i in range(0, len(cur) - 1, 2):
                        t = smp.tile(
                            [100, 4 * NF], bf16, tag=f"{tag}{k}", name=f"{tag}{k}"
                        )
                        k += 1
                        tv = t[:].rearrange("p (k j) -> p k j", k=4)
                        nc.vector.tensor_tensor(tv, cur[i], cur[i + 1], op=OP.add)
                        nxt.append(tv)
                    if len(cur) % 2:
                        nxt.append(cur[-1])
                    cur = nxt
                return cur[0]

            lp = tree_sum(npos, 0, "lp")
            ln = tree_sum(nneg, npos, "ln")
            ell = smp.tile([100, 4 * NF], bf16, tag="ell")
            ellv = ell[:].rearrange("p (k j) -> p k j", k=4)
            nc.vector.tensor_tensor(ellv, lp, ln, op=OP.subtract)
            ellm = smp.tile([100, 4 * NF], bf16, tag="ellm")
            ellmv = ellm[:].rearrange("p (k j) -> p k j", k=4)
            mb = maskt[:].unsqueeze(1).broadcast_to([100, 4, NF])
            nc.vector.tensor_tensor(ellmv, ellv, mb, op=OP.add)

            e_t = smp.tile([100, 4 * NF], bf16, tag="et")
            nc.scalar.activation(e_t[:], ellm[:], AF.Exp)
            ers = smp.tile([100, 4 * NF], bf16, tag="ers")
            nc.vector.tensor_tensor(ers[:], e_t[:], rr4[:], op=OP.mult)
            # per-k contiguous reduces (strided/multi-dim reduce faults HW)
            for kk in range(4):
                nc.vector.reduce_sum(
                    acc[:, NIT + base_it + kk : NIT + base_it + kk + 1],
                    e_t[:, kk * NF : (kk + 1) * NF],
                    axis=mybir.AxisListType.X,
                )
                nc.vector.reduce_sum(
                    acc[:, base_it + kk : base_it + kk + 1],
                    ers[:, kk * NF : (kk + 1) * NF],
                    axis=mybir.AxisListType.X,
                )

        # final: per-batch partition sums (num, den), divide, store
        pf = psf.tile([2, 2 * NIT], f32)
        nc.tensor.matmul(pf[:], onesfin[:], acc[:], start=True, stop=True)
        rcp = smp.tile([2, NIT], f32, tag="rcp")
        nc.vector.reciprocal(rcp[:], pf[:, NIT : 2 * NIT])
        res = smp.tile([2, NIT], f32, tag="res")
        nc.vector.tensor_tensor(res[:], pf[:, 0:NIT], rcp[:], op=OP.mult)
        out_v = out_ext[:].rearrange("(i m) o -> m (i o)", m=2)
        nc.sync.dma_start(out_v, res[:])

    nc.compile()
    return nc


def _host_prep(x, attention_W, attention_b, projection_h, projection_p):
    import ml_dtypes

    bf = ml_dtypes.bfloat16
    x = np.ascontiguousarray(np.asarray(x, dtype=np.float32))
    W = np.asarray(attention_W, dtype=np.float32)
    bv = np.asarray(attention_b, dtype=np.float32)
    h = np.asarray(projection_h, dtype=np.float32).reshape(-1)
    p = np.asarray(projection_p, dtype=np.float32).reshape(-1)

    habs = np.abs(h)
    sgn = np.sign(h)
    order = np.argsort(-sgn, kind="stable")
    Wp = (W * habs[None, :])[:, order]
    bp = (bv * habs)[order]
    npos = int((sgn[order] > 0).sum())

    Cmat = np.concatenate([Wp, p[:, None]], axis=1)  # [64, 11]
    cexp64 = np.broadcast_to(Cmat.T[:, None, :], (NCOL, NF, E))  # [11,50,64]
    cexp64 = np.transpose(cexp64, (2, 0, 1)).reshape(E, W5)  # [64,550]
    cexp = np.concatenate([cexp64, cexp64], axis=0).astype(bf)  # [128,550]
    cbias = (
        np.concatenate([np.repeat(bp, NF), np.zeros(NF)]).reshape(1, -1).astype(bf)
    )
    im = np.arange(NF)
    m50 = np.where(im[:, None] < im[None, :], 0.0, MASK_NEG).astype(bf)
    maskt = np.ascontiguousarray(np.concatenate([m50, m50], axis=0))  # [100, 50]
    onesfin = np.zeros((100, 2), dtype=np.float32)
    onesfin[0:50, 0] = 1.0
    onesfin[50:100, 1] = 1.0

    # per-core packed layouts
    xcores = x.reshape(NCORES, B_LOC, NF, E)
    xt_bf = np.transpose(xcores, (0, 1, 3, 2)).astype(bf)  # [C, 256, 64, 50]
    xt_it = xt_bf.reshape(NCORES, NIT, 2, E, NF)  # [C, it, half, e, j]
    # xt [128, NIT*50]: rows (half, e), cols (it, j)
    xt_all = np.ascontiguousarray(
        xt_it.transpose(0, 2, 3, 1, 4).reshape(NCORES, 128, NIT * NF)
    )
    # std [128, NIT*100]: block-diag per iteration
    std_all = np.zeros((NCORES, 2, E, NIT, 2, NF), dtype=bf)  # [C,rh,e,it,ch,j]
    std_all[:, 0, :, :, 0, :] = xt_it[:, :, 0].transpose(0, 2, 1, 3)
    std_all[:, 1, :, :, 1, :] = xt_it[:, :, 1].transpose(0, 2, 1, 3)
    std_all = np.ascontiguousarray(std_all.reshape(NCORES, 128, NIT * 100))

    return npos, xt_all, std_all, cexp, cbias, maskt, onesfin


def kernel(x, attention_W, attention_b, projection_h, projection_p):
    global LAST_RESULTS
    from concourse.bass_utils import run_bass_kernel_spmd

    npos, xt_all, std_all, cexp, cbias, maskt, onesfin = _host_prep(
        x, attention_W, attention_b, projection_h, projection_p
    )
    nc = _build(npos, NA - npos)

    in_maps = []
    for c in range(NCORES):
        in_maps.append(
            {
                "xt": xt_all[c],
                "std": std_all[c],
                "cexp": cexp,
                "cbias": cbias,
                "maskt": maskt,
                "onesfin": onesfin,
            }
        )
    trace = os.environ.get("BASS_KERNEL_TRACE", "0") == "1"
    res = run_bass_kernel_spmd(nc, in_maps, core_ids=list(range(NCORES)), trace=trace)
    LAST_RESULTS = res
    outs = [np.asarray(r["out"]).reshape(B_LOC, 1) for r in res.results]
    return np.concatenate(outs, axis=0).astype(np.float32)
